# revision 16
# baseline (speedup 1.0000x reference)
"""Trainium2 Bass kernel for nn_CCM: per-pixel complex 3x3 conv mask.

Math (per batch element b, sharded 1 batch element per NeuronCore):
  y[t,f] = sum_{c=0..26} m[c,t,f] * (w_{k(c)} * X)[t+i(c)-2, f+j(c)-1]
where c = 9*k + 3*i + j, w_k = v[0,k] + 1j*v[1,k] (cube roots of unity),
X = xr + 1j*xi, zero padded (causal in t: 2 top; symmetric in f: 1,1).

Layout: t = 8*p + tau, partitions p in [0,125), (tau, f) in the free dim.
U planes are stored as complex PAIRS [TP, NS, 2, FP] fp16 so one DVE
TensorTensor handles both real and imag of a tap (m broadcast stride-0
across the comp dim). All fp16 tensors use even-element row strides
(FP=260, FW=258) so every innermost run is 4-byte aligned -> DVE 2x_1p
perf mode engages (2 elem/cycle/lane). Center taps (df=0) would start at
an odd column, so k=0 center taps read a pre-shifted copy B0; the k=1
center taps go to Pool (alignment-agnostic); c=25 stays 1x on DVE.
PSUM->SBUF copies are merged 8-slots-at-a-time to cut ACT overhead.
"""

import sys
import numpy as np

sys.path.insert(0, "/opt/trn_rl_repo")

B = 8
C = 27
T = 1000
F = 257
TP = 125          # partitions
TAU = 8           # t = 8*p + tau
NS = 10           # tau slots in U planes: tau in [-2, 8)
FP = 260          # padded f width (f in [-1, 258); col 259 = alignment pad)
FW = 258          # row width of m16/pr/acc tiles (col 257 = alignment pad)
SQ3H = float(np.sqrt(3.0) / 2.0)

POOL_TAPS = (10, 13, 16, 19, 22)   # on Pool: k=1 centers + 2 k=2 centers
# DVE compute order: k0 (B0-aligned centers), then k1, k2 leftovers
DVE_TAPS = [0, 1, 2, 3, 4, 5, 6, 7, 8, 9, 11, 12, 14, 15, 17, 18, 20, 21, 23, 24, 25, 26]
B0_TAPS = (1, 4, 7)                # k=0 centers read shifted plane B0
# DMA order: feeds DVE's k0 burst early, pool taps just in time
DMA_ORDER = [0, 10, 1, 2, 3, 4, 5, 13, 6, 7, 8, 16, 9, 19, 11, 12, 14, 15, 22,
             17, 18, 20, 21, 23, 24, 25, 26]

_CACHE = {}


def _emit(ctx, tc, m_ap, x_ap, id_ap, y_ap):
    import concourse.mybir as mybir

    nc = tc.nc
    f32 = mybir.dt.float32
    f16 = mybir.dt.float16
    FCS = [(0, 128), (128, 128), (256, 1)]  # f chunks for transposes

    const = ctx.enter_context(tc.tile_pool(name="const", bufs=1))
    planes = ctx.enter_context(tc.tile_pool(name="planes", bufs=1))
    mpool = ctx.enter_context(tc.tile_pool(name="mtiles", bufs=4))
    m16d = ctx.enter_context(tc.tile_pool(name="m16d", bufs=4))
    m16p = ctx.enter_context(tc.tile_pool(name="m16p", bufs=5))
    work = ctx.enter_context(tc.tile_pool(name="work", bufs=2))
    pwork = ctx.enter_context(tc.tile_pool(name="pwork", bufs=1))
    xstage = ctx.enter_context(tc.tile_pool(name="xstage", bufs=2))
    psum = ctx.enter_context(tc.tile_pool(name="psum", bufs=2, space="PSUM"))

    ident = const.tile([128, 128], f32, tag="ident")
    nc.sync.dma_start(ident[:], id_ap)
    ident16 = const.tile([128, 128], f16, tag="ident16")
    nc.scalar.copy(ident16[:], ident[:])

    # ---- U plane pair tiles; u0 memset for pad cols (0, 258, 259)
    u0 = planes.tile([TP, NS, 2, FP], f16, tag="u0")
    u1 = planes.tile([TP, NS, 2, FP], f16, tag="u1")
    u2 = planes.tile([TP, NS, 2, FP], f16, tag="u2")
    b0 = planes.tile([TP, NS, 2, FP], f16, tag="b0")
    nc.gpsimd.memset(u0[:], 0.0)

    # ---- load x in natural layout [f, (tt, comp)] (tt = t + 2), convert fp16
    xn16s = []
    for (f0, fw) in FCS:
        xn = xstage.tile([128, (T + 2) * 2], f32, tag="xn", name="xn")[0:fw]
        nc.vector.memset(xn[:, 0:4], 0.0)
        nc.sync.dma_start(
            xn[:, 4:], x_ap[f0:f0 + fw].rearrange("f t c -> f (t c)")
        )
        xn16 = const.tile([fw, (T + 2) * 2], f16, tag=f"xn16_{f0}")
        nc.scalar.copy(xn16[:], xn[:])
        xn16s.append(xn16)

    # ---- m DMAs
    mtiles = {}
    for c in DMA_ORDER:
        mt = mpool.tile([TP, TAU * F], f32, tag="mt", name="mt")
        nc.sync.dma_start(mt[:], m_ap[c].rearrange("(p t) f -> p (t f)", p=TP))
        mtiles[c] = mt

    # ---- transpose x into u0 [TP, NS, 2, FP]; merged 8-slot PSUM copies
    for q in range(2):
        for ci, (f0, fw) in enumerate(FCS):
            xn3 = xn16s[ci].rearrange("f (t c) -> f t c", c=2)
            tp8 = psum.tile([TP, TAU, 128], f16, tag="tp8", name="tp8")
            for ts in range(TAU):
                nc.tensor.transpose(
                    tp8[0:TP, ts, 0:fw],
                    xn3[0:fw, ts:ts + TAU * (TP - 1) + 1:TAU, q],
                    ident16[0:fw, 0:fw],
                )
            nc.scalar.copy(u0[:, 0:TAU, q, 1 + f0:1 + f0 + fw], tp8[:, :, 0:fw])
            tp2 = psum.tile([TP, 2, 128], f16, tag="tp2", name="tp2")
            for ts in range(TAU, NS):
                nc.tensor.transpose(
                    tp2[0:TP, ts - TAU, 0:fw],
                    xn3[0:fw, ts:ts + TAU * (TP - 1) + 1:TAU, q],
                    ident16[0:fw, 0:fw],
                )
            nc.scalar.copy(u0[:, TAU:NS, q, 1 + f0:1 + f0 + fw], tp2[:, :, 0:fw])

    # ---- U planes: U_1 = w_1*X, U_2 = w_2*X (w = -1/2 +- i*sqrt(3)/2)
    x_r = u0[:, :, 0, :]
    x_i = u0[:, :, 1, :]
    t1 = planes.tile([TP, NS, FP], f16, tag="t1")
    t2 = planes.tile([TP, NS, FP], f16, tag="t2")
    nc.vector.tensor_scalar_mul(t1[:], x_i, SQ3H)            # s*xi
    nc.vector.tensor_scalar_mul(t2[:], x_r, SQ3H)            # s*xr
    nc.vector.tensor_scalar_mul(u1[:, :, 0, :], x_r, -0.5)   # -xr/2
    nc.vector.tensor_scalar_mul(u1[:, :, 1, :], x_i, -0.5)   # -xi/2
    nc.vector.tensor_sub(u2[:, :, 0, :], u1[:, :, 0, :], t1[:])
    nc.vector.tensor_add(u1[:, :, 0, :], u1[:, :, 0, :], t1[:])
    nc.vector.tensor_add(u2[:, :, 1, :], u1[:, :, 1, :], t2[:])
    nc.vector.tensor_sub(u1[:, :, 1, :], u1[:, :, 1, :], t2[:])
    # After the in-place updates:
    #   u2.re = -xr/2 - s*xi (U_1 real),  u2.im = -xi/2 + s*xr (U_1 imag)
    #   u1.re = -xr/2 + s*xi (U_2 real),  u1.im = -xi/2 - s*xr (U_2 imag)
    # so the k->plane map is U[1] = u2, U[2] = u1.
    U = [u0, u2, u1]

    # ---- B0: u0 shifted left one column so df=0 taps start 4B-aligned
    nc.gpsimd.tensor_copy(b0[:, :, :, 0:FW], u0[:, :, :, 1:FP - 1])

    # ---- tap loops (paired complex ops; m broadcast across comp dim)
    acc = planes.tile([TP, TAU, 2, FW], f16, tag="acc")
    pacc = planes.tile([TP, TAU, 2, FW], f16, tag="pacc")

    def u_slice(c):
        kk, n = divmod(c, 9)
        i, j = divmod(n, 3)
        dt, df = i - 2, j - 1
        if c in B0_TAPS:
            return b0[:, dt + 2:dt + 2 + TAU, :, 0:F]
        return U[kk][:, dt + 2:dt + 2 + TAU, :, df + 1:df + 1 + F]

    def m_bcast(m16):
        return m16[:, :, 0:F].unsqueeze(2).broadcast_to((TP, TAU, 2, F))

    # Pool chain (k=1 centers + 19, 22): all fp16
    firstp = True
    for c in POOL_TAPS:
        m16 = m16p.tile([TP, TAU, FW], f16, tag="m16p", name="m16p")
        nc.scalar.copy(m16[:, :, 0:F], mtiles[c].rearrange("p (t f) -> p t f", f=F))
        if firstp:
            nc.gpsimd.tensor_mul(pacc[:, :, :, 0:F], m_bcast(m16), u_slice(c))
            firstp = False
        else:
            pr = pwork.tile([TP, TAU, 2, FW], f16, tag="pprod", name="pprod")
            nc.gpsimd.tensor_mul(pr[:, :, :, 0:F], m_bcast(m16), u_slice(c))
            nc.gpsimd.tensor_add(pacc[:, :, :, 0:F], pacc[:, :, :, 0:F],
                                 pr[:, :, :, 0:F])

    # DVE chain: stride-0 broadcast kills DVE throughput, so each tap does
    # two plain 3D muls (real/imag, both 2x-aligned) + one paired add.
    firstd = True
    for c in DVE_TAPS:
        m16 = m16d.tile([TP, TAU, FW], f16, tag="m16d", name="m16d")
        nc.scalar.copy(m16[:, :, 0:F], mtiles[c].rearrange("p (t f) -> p t f", f=F))
        us = u_slice(c)
        dst = acc if firstd else work.tile([TP, TAU, 2, FW], f16, tag="prod",
                                           name="prod")
        nc.vector.tensor_mul(dst[:, :, 0, 0:F], m16[:, :, 0:F], us[:, :, 0, :])
        nc.vector.tensor_mul(dst[:, :, 1, 0:F], m16[:, :, 0:F], us[:, :, 1, :])
        if firstd:
            firstd = False
        else:
            nc.vector.tensor_add(acc[:, :, :, 0:F], acc[:, :, :, 0:F],
                                 dst[:, :, :, 0:F])

    # ---- combine DVE + Pool accumulators in place
    nc.vector.tensor_add(acc[:, :, :, 0:F], acc[:, :, :, 0:F], pacc[:, :, :, 0:F])

    # ---- transpose back to [f, (t, comp)]; merged 8-tau PSUM copies
    for ci, (f0, fw) in enumerate(FCS):
        yo = xstage.tile([128, T * 2], f32, tag="yo", name="yo")[0:fw]
        yv = yo.rearrange("f (p s c) -> f s p c", s=TAU, c=2)
        for comp in range(2):
            tpo = psum.tile([128, TAU, 126], f16, tag="tpo", name="tpo")
            for ts in range(TAU):
                nc.tensor.transpose(
                    tpo[0:fw, ts, 0:TP], acc[:, ts, comp, f0:f0 + fw],
                    ident16[0:TP, 0:TP],
                )
            nc.scalar.copy(yv[0:fw, :, :, comp], tpo[0:fw, :, 0:TP])
        nc.sync.dma_start(y_ap[f0:f0 + fw].rearrange("f t c -> f (t c)"), yo[:])


def _build():
    if "nc" in _CACHE:
        return _CACHE["nc"]
    from contextlib import ExitStack
    from concourse import bacc, mybir
    import concourse.tile as tile

    f32 = mybir.dt.float32
    nc = bacc.Bacc("TRN2", target_bir_lowering=False, debug=False, num_devices=B)
    m_d = nc.dram_tensor("m", (C, T, F), f32, kind="ExternalInput")
    x_d = nc.dram_tensor("x", (F, T, 2), f32, kind="ExternalInput")
    id_d = nc.dram_tensor("ident", (128, 128), f32, kind="ExternalInput")
    y_d = nc.dram_tensor("y", (F, T, 2), f32, kind="ExternalOutput")

    with tile.TileContext(nc) as tc:
        with ExitStack() as ctx:
            _emit(ctx, tc, m_d.ap(), x_d.ap(), id_d.ap(), y_d.ap())
    nc.compile()
    _CACHE["nc"] = nc
    return nc


def _in_maps(m, x):
    ident = np.eye(128, dtype=np.float32)
    return [
        {"m": np.ascontiguousarray(m[b]), "x": np.ascontiguousarray(x[b]),
         "ident": ident}
        for b in range(B)
    ]


def kernel(m, x, v, _trace=False):
    from concourse import bass_utils

    m = np.asarray(m, dtype=np.float32)
    x = np.asarray(x, dtype=np.float32)
    nc = _build()
    res = bass_utils.run_bass_kernel_spmd(
        nc, _in_maps(m, x), core_ids=list(range(B)), trace=_trace
    )
    kernel.last_results = res
    y = np.stack([res.results[b]["y"] for b in range(B)], axis=0)
    return y
